# revision 1
# baseline (speedup 1.0000x reference)
"""Trainium2 Bass kernel for nn_BasicNet4 (Emformer encoder, sparse attention).

Strategy:
  - Data-parallel over batch B=8 across 8 NeuronCores (weights replicated).
  - Tokens reordered host-side into segment-interleaved order:
    seg i -> [rc_i, u_{4i}, u_{4i+1}, u_{4i+2}, u_{4i+3}]  (5 tokens x 256 segs = 1280)
    so attention is block-diagonal with 5x5 blocks.
  - Activations kept transposed in SBUF: [d on partitions (4 tiles of 128), tokens on free].
  - LayerNorm gains/biases folded into weights host-side; LN stats computed with
    ones-matmul partition reductions on the PE (broadcast form).
  - Attention masks folded into the score matmuls as extra low-rank (+/-C indicator)
    contraction terms; block-diagonal attention computed per 128-token diagonal tile
    plus small "halo" edge strips.
  - bf16 matmul operands / residual stream, fp32 PSUM accumulation.
"""

import sys

sys.path.insert(0, "/opt/trn_rl_repo")

import numpy as np
import ml_dtypes

import concourse.bass as bass
import concourse.mybir as mybir
import concourse.tile as tile
from concourse import bass_utils, bacc

bf16 = ml_dtypes.bfloat16
dt = mybir.dt
AF = mybir.ActivationFunctionType
ALU = mybir.AluOpType

# Model config (hardcoded from the problem spec)
D, H, FFN, L = 512, 4, 128, 4
SEG, RC = 4, 1
B, T = 8, 1025
U = T - RC            # 1024
NSEG = U // SEG       # 256
TT = NSEG * (SEG + RC)  # 1280 interleaved tokens
NT = TT // 128        # 10 token tiles
DT = D // 128         # 4 d tiles
DH = D // H           # 128 (= one partition tile per head)
NCORES = 8
CHUNKS = [(0, 512), (512, 512), (1024, 256)]  # free-dim chunks <= 512 (one PSUM bank)

CBF = np.float32(bf16(np.float32(1e9)))  # mask constant, exactly representable in bf16

_COMPILED = None


def _tok_index():
    # interleaved token t -> original frame index in x[:, :T]
    t = np.arange(TT)
    seg = t // 5
    pos = t % 5
    off = np.array([4, 0, 1, 2, 3])[pos]
    return 4 * seg + off  # in [0, 1024]


def _qt_geometry(qt):
    """MID window is the aligned [128qt, 128qt+128). LEFT/RIGHT edges are the
    few extra k-tokens of the straddling segments."""
    q0 = 128 * qt
    sk = 5 * (q0 // 5)
    op = q0 - sk                      # 0..4
    ek = min(5 * (-(-(q0 + 128) // 5)), TT)
    nL = op                           # left edge width (tokens [sk, q0))
    nR = max(ek - (q0 + 128), 0)      # right edge width (tokens [q0+128, ek))
    return q0, sk, op, nL, nR


def _mask_consts():
    """Per-qt mask matmul operands (host-computed, bf16).
    MID:  Lmid[qt] [128,128] (lhsT), Rmid[qt] [128,128] (rhs):
          sum_r Lmid[r,k]*Rmid[r,j] = -C + C*[seg(k)==seg(j)]  (window-local segs)
    EDGE: Lel[qt] [128,8], Rel[qt] [128,128]: same for the 8 edge slots
          (slots 0..3 = LEFT tokens, 4..7 = RIGHT tokens; invalid slots -> -C only).
    """
    Lmid = np.zeros((NT, 128, 128), np.float32)
    Rmid = np.zeros((NT, 128, 128), np.float32)
    Lel = np.zeros((NT, 128, 8), np.float32)
    Rel = np.zeros((NT, 128, 128), np.float32)
    for qt in range(NT):
        q0, sk, op, nL, nR = _qt_geometry(qt)
        segq = (op + np.arange(128)) // 5       # window-local seg of q (and mid k)
        # MID
        Lmid[qt, 0, :] = 1.0
        Rmid[qt, 0, :] = -CBF
        nseg = segq[-1] + 1
        for i in range(nseg):
            Lmid[qt, 1 + i, :] = (segq == i)
            Rmid[qt, 1 + i, :] = CBF * (segq == i)
        # EDGE
        Lel[qt, 0, :] = 1.0
        Rel[qt, 0, :] = -CBF
        slot_seg = np.full(8, -1)
        for s in range(nL):
            slot_seg[s] = 0                      # left tokens are in window-local seg 0
        for s in range(nR):
            slot_seg[4 + s] = (128 + op + s) // 5
        esegs = sorted(set(slot_seg[slot_seg >= 0]))
        for j, e in enumerate(esegs):
            Lel[qt, 1 + j, :] = (slot_seg == e)
            Rel[qt, 1 + j, :] = CBF * (segq == e)
    return Lmid.astype(bf16), Rmid.astype(bf16), Lel.astype(bf16), Rel.astype(bf16)


def _host_prep(ins):
    """Fold LN affines into weights, transpose, cast. Returns shared input map."""
    f32 = np.float32
    m = {}
    scale = np.float32(DH) ** -0.5
    for l in range(L):
        g_i, b_i = f32(ins["ln_in_g"][l]), f32(ins["ln_in_b"][l])
        g_f, b_f = f32(ins["ff_ln_g"][l]), f32(ins["ff_ln_b"][l])
        Wq = f32(ins["Wq"][l]);  bq = f32(ins["bq"][l])
        Wk = f32(ins["Wkv"][l][:D]);  bk = f32(ins["bkv"][l][:D])
        Wv = f32(ins["Wkv"][l][D:]);  bv = f32(ins["bkv"][l][D:])
        Wo = f32(ins["Wo"][l]);  bo = f32(ins["bo"][l])
        W1 = f32(ins["W1"][l]);  b1 = f32(ins["b1"][l])
        W2 = f32(ins["W2"][l]);  b2 = f32(ins["b2"][l])
        Wq_ = scale * (Wq * g_i[None, :]); bq_ = scale * (bq + Wq @ b_i)
        Wk_ = Wk * g_i[None, :];           bk_ = bk + Wk @ b_i
        Wv_ = Wv * g_i[None, :];           bv_ = bv + Wv @ b_i
        W1_ = W1 * g_f[None, :];           b1_ = b1 + W1 @ b_f
        m[f"wq{l}"] = Wq_.T.copy().astype(bf16)   # [din, dout]
        m[f"wk{l}"] = Wk_.T.copy().astype(bf16)
        m[f"wv{l}"] = Wv_.T.copy().astype(bf16)
        m[f"wo{l}"] = Wo.T.copy().astype(bf16)
        m[f"w1{l}"] = W1_.T.copy().astype(bf16)   # [512, 128]
        m[f"w2{l}"] = W2.T.copy().astype(bf16)    # [128, 512]
        m[f"bq{l}"] = bq_.reshape(DT, 128).T.copy()       # [128, DT] f32 per-partition
        m[f"bk{l}"] = bk_.reshape(DT, 128).T.copy()
        m[f"bv{l}"] = bv_.reshape(1, D).astype(bf16)      # [1, 512] row (K=1 matmul)
        m[f"bo{l}"] = bo.reshape(DT, 128).T.copy()
        m[f"b1{l}"] = b1_.reshape(1, 128).T.copy()        # [128, 1]
        m[f"b2{l}"] = b2.reshape(DT, 128).T.copy()
        m[f"go{l}"] = f32(ins["ln_out_g"][l]).reshape(DT, 128).T.copy()
        m[f"bo2{l}"] = f32(ins["ln_out_b"][l]).reshape(DT, 128).T.copy()
    Lmid, Rmid, Lel, Rel = _mask_consts()
    m["lmid"] = np.ascontiguousarray(Lmid.transpose(1, 0, 2))  # [128, NT, 128]
    m["rmid"] = np.ascontiguousarray(Rmid.transpose(1, 0, 2))
    m["lel"] = np.ascontiguousarray(Lel.transpose(1, 0, 2))    # [128, NT, 8]
    m["rel"] = np.ascontiguousarray(Rel.transpose(1, 0, 2))
    m["ones_c"] = np.full((128, 128), 1.0 / D, bf16)           # stats lhsT (bcast reduce)
    m["allones"] = np.ones((128, 128), bf16)                   # denominator lhsT
    m["ones1"] = np.ones((1, 128), bf16)                       # K=1 bcast lhsT
    m["ident"] = np.eye(128, dtype=bf16)                       # residual adds
    return m


def _dram_inputs(nc):
    a = {}
    def inp(name, shape, dtype):
        a[name] = nc.dram_tensor(name, list(shape), dtype, kind="ExternalInput").ap()
    inp("xT", (D, TT), dt.bfloat16)
    for l in range(L):
        inp(f"wq{l}", (D, D), dt.bfloat16); inp(f"wk{l}", (D, D), dt.bfloat16)
        inp(f"wv{l}", (D, D), dt.bfloat16); inp(f"wo{l}", (D, D), dt.bfloat16)
        inp(f"w1{l}", (D, FFN), dt.bfloat16); inp(f"w2{l}", (FFN, D), dt.bfloat16)
        inp(f"bq{l}", (128, DT), dt.float32); inp(f"bk{l}", (128, DT), dt.float32)
        inp(f"bv{l}", (1, D), dt.bfloat16); inp(f"bo{l}", (128, DT), dt.float32)
        inp(f"b1{l}", (128, 1), dt.float32); inp(f"b2{l}", (128, DT), dt.float32)
        inp(f"go{l}", (128, DT), dt.float32); inp(f"bo2{l}", (128, DT), dt.float32)
    inp("lmid", (128, NT, 128), dt.bfloat16); inp("rmid", (128, NT, 128), dt.bfloat16)
    inp("lel", (128, NT, 8), dt.bfloat16); inp("rel", (128, NT, 128), dt.bfloat16)
    inp("ones_c", (128, 128), dt.bfloat16); inp("allones", (128, 128), dt.bfloat16)
    inp("ones1", (1, 128), dt.bfloat16); inp("ident", (128, 128), dt.bfloat16)
    out = nc.dram_tensor("out", [128, DT], dt.float32, kind="ExternalOutput").ap()
    return a, out


def _ln_normalize(nc, acts, sbufs, psums, smalls, cat, z_out, eps_tile):
    """z = (cat - mean) * rstd in bcast form. cat/z: [128, DT, TT] bf16 sbuf."""
    ones_c = smalls["ones_c"]
    # squares on GPSIMD (bf16)
    sq = acts.tile([128, DT, TT], dt.bfloat16, tag="sq")
    for d in range(DT):
        nc.gpsimd.tensor_tensor(sq[:, d], cat[:, d], cat[:, d], ALU.mult)
    p_mu = psums.tile([128, TT], dt.float32, tag="big")
    p_e2 = psums.tile([128, TT], dt.float32, tag="big")
    for (c0, cn) in CHUNKS:
        for d in range(DT):
            nc.tensor.matmul(p_mu[:, c0:c0 + cn], ones_c[:], cat[:, d, c0:c0 + cn],
                             start=(d == 0), stop=(d == DT - 1))
        for d in range(DT):
            nc.tensor.matmul(p_e2[:, c0:c0 + cn], ones_c[:], sq[:, d, c0:c0 + cn],
                             start=(d == 0), stop=(d == DT - 1))
    # Note ones_c = 1/512 -> p_mu = mean, p_e2 = E[x^2]; all rows identical (allones trick
    # not needed: ones_c is [128,1] so out partition count is... see below)
    mu_b = sbufs.tile([128, TT], dt.bfloat16, tag="mu")
    sq_mu = sbufs.tile([128, TT], dt.float32, tag="sqmu")
    var = sbufs.tile([128, TT], dt.float32, tag="var")
    A = sbufs.tile([128, TT], dt.bfloat16, tag="A")
    nc.scalar.activation(sq_mu[:], p_mu[:], AF.Square)
    nc.vector.tensor_copy(mu_b[:], p_mu[:])
    nc.vector.tensor_tensor(var[:], p_e2[:], sq_mu[:], ALU.subtract)
    nc.scalar.activation(var[:], var[:], AF.Sqrt, bias=eps_tile[:], scale=1.0)
    with nc.allow_low_precision(reason="per-token rstd in bf16 is fine here"):
        nc.vector.reciprocal(A[:], var[:])
    for d in range(DT):
        z1 = sbufs.tile([128, TT], dt.bfloat16, tag="z1")
        nc.vector.tensor_tensor(z1[:], cat[:, d], mu_b[:], ALU.subtract)
        nc.vector.tensor_tensor(z_out[:, d], z1[:], A[:], ALU.mult)


def _trace(nc):
    a, out_dram = _dram_inputs(nc)
    with tile.TileContext(nc) as tc:
        import contextlib
        ctx = contextlib.ExitStack()
        with ctx:
            consts = ctx.enter_context(tc.tile_pool(name="consts", bufs=1))
            wpool = ctx.enter_context(tc.tile_pool(name="w", bufs=2))
            acts = ctx.enter_context(tc.tile_pool(name="acts", bufs=1))
            sbufs = ctx.enter_context(tc.tile_pool(name="sbufs", bufs=2))
            psums = ctx.enter_context(tc.tile_pool(name="psums", bufs=2, space="PSUM"))
            pv = ctx.enter_context(tc.tile_pool(name="pv", bufs=2, space="PSUM"))

            # constants
            smalls = {}
            for name, shape, dd in [
                ("lmid", [128, NT, 128], dt.bfloat16), ("rmid", [128, NT, 128], dt.bfloat16),
                ("lel", [128, NT, 8], dt.bfloat16), ("rel", [128, NT, 128], dt.bfloat16),
                ("ones_c", [128, 128], dt.bfloat16), ("allones", [128, 128], dt.bfloat16),
                ("ones1", [1, 128], dt.bfloat16), ("ident", [128, 128], dt.bfloat16),
            ]:
                t = consts.tile(shape, dd, tag=name)
                nc.sync.dma_start(t[:], a[name])
                smalls[name] = t
            eps_tile = consts.tile([128, 1], dt.float32)
            nc.vector.memset(eps_tile[:], 1e-5)

            # initial residual stream (transposed, interleaved)
            cat = acts.tile([128, DT, TT], dt.bfloat16, tag="cat0")
            nc.sync.dma_start(cat[:], a["xT"].rearrange("(dtile p) t -> p dtile t", p=128))

            for l in range(L):
                # --- load layer weights ---
                w = {}
                for nm, shape in [("wq", [128, DT, D]), ("wk", [128, DT, D]),
                                  ("wv", [128, DT, D]), ("wo", [128, DT, D]),
                                  ("w1", [128, DT, FFN]), ("w2", [128, D])]:
                    t = wpool.tile(shape, dt.bfloat16, tag=nm)
                    src = a[f"{nm}{l}"]
                    if nm == "w2":
                        nc.sync.dma_start(t[:], src)
                    else:
                        nc.sync.dma_start(t[:], src.rearrange("(dtile p) o -> p dtile o", p=128))
                    w[nm] = t
                bias = {}
                for nm in ["bq", "bk", "bo", "b1", "b2", "go", "bo2"]:
                    t = wpool.tile([128, DT] if nm != "b1" else [128, 1], dt.float32, tag=nm)
                    nc.sync.dma_start(t[:], a[f"{nm}{l}"])
                    bias[nm] = t
                bv = wpool.tile([1, D], dt.bfloat16, tag="bv")
                nc.sync.dma_start(bv[:], a[f"bv{l}"])

                # --- ln_in -> z ---
                z = acts.tile([128, DT, TT], dt.bfloat16, tag="z")
                _ln_normalize(nc, acts, sbufs, psums, smalls, cat, z, eps_tile)

                # --- Q, K projections (weights stationary -> transposed out) ---
                qk = {}
                for nm, bnm in [("wq", "bq"), ("wk", "bk")]:
                    dst = acts.tile([128, DT, TT], dt.bfloat16, tag="q" if nm == "wq" else "k")
                    for o in range(DT):
                        p = psums.tile([128, TT], dt.float32, tag="big")
                        for (c0, cn) in CHUNKS:
                            for d in range(DT):
                                nc.tensor.matmul(
                                    p[:, c0:c0 + cn],
                                    w[nm][:, d, 128 * o:128 * o + 128],
                                    z[:, d, c0:c0 + cn],
                                    start=(d == 0), stop=(d == DT - 1))
                        nc.scalar.activation(dst[:, o], p[:], AF.Identity,
                                             bias=bias[bnm][:, o:o + 1], scale=1.0)
                    qk[nm] = dst
                q_t, k_t = qk["wq"], qk["wk"]

                # --- V projection (acts stationary -> natural out [t, d]) ---
                v_nat = acts.tile([128, NT, D], dt.bfloat16, tag="v")
                for tt_i in range(NT):
                    p = pv.tile([128, D], dt.float32, tag="small")
                    for d in range(DT):
                        nc.tensor.matmul(p[:], z[:, d, 128 * tt_i:128 * tt_i + 128],
                                         w["wv"][:, d, :], start=(d == 0), stop=False)
                    nc.tensor.matmul(p[:], smalls["ones1"][:, 128 * 0:128], bv[:],
                                     start=False, stop=True)
                    nc.vector.tensor_copy(v_nat[:, tt_i], p[:])

                # --- halos (K columns easy; V rows via partition-offset copies) ---
                k_halo = sbufs.tile([128, DT, NT, 8], dt.bfloat16, tag="khalo")
                v_halo = acts.tile([8, NT, D], dt.bfloat16, tag="vhalo")
                nc.gpsimd.memset(k_halo[:], 0.0)
                nc.gpsimd.memset(v_halo[:], 0.0)
                for qt in range(NT):
                    q0, sk, op, nL, nR = _qt_geometry(qt)
                    if nL > 0:
                        nc.gpsimd.tensor_copy(k_halo[:, :, qt, 0:nL], k_t[:, :, sk:sk + nL])
                        nc.sync.dma_start(v_halo[0:nL, qt, :],
                                          v_nat[128 - nL:128, qt - 1, :])
                    if nR > 0:
                        nc.gpsimd.tensor_copy(k_halo[:, :, qt, 4:4 + nR],
                                              k_t[:, :, q0 + 128:q0 + 128 + nR])
                        nc.sync.dma_start(v_halo[4:4 + nR, qt, :],
                                          v_nat[0:nR, qt + 1, :])

                # --- attention per head ---
                attn = acts.tile([128, DT, TT], dt.bfloat16, tag="attn")
                for h in range(H):
                    p_mid = psums.tile([128, TT], dt.float32, tag="big")
                    p_edge = psums.tile([128, TT], dt.float32, tag="big")
                    for qt in range(NT):
                        q0 = 128 * qt
                        qs = q_t[:, h, q0:q0 + 128]
                        nc.tensor.matmul(p_mid[:, q0:q0 + 128], smalls["lmid"][:, qt],
                                         smalls["rmid"][:, qt], start=True, stop=False)
                        nc.tensor.matmul(p_mid[:, q0:q0 + 128], k_t[:, h, q0:q0 + 128],
                                         qs, start=False, stop=True)
                        nc.tensor.matmul(p_edge[0:8, q0:q0 + 128], smalls["lel"][:, qt],
                                         smalls["rel"][:, qt], start=True, stop=False)
                        nc.tensor.matmul(p_edge[0:8, q0:q0 + 128], k_halo[:, h, qt], qs,
                                         start=False, stop=True)
                    pa = sbufs.tile([128, TT], dt.bfloat16, tag="pa")
                    pe = sbufs.tile([8, TT], dt.bfloat16, tag="pe")
                    nc.scalar.activation(pa[:], p_mid[:], AF.Exp)
                    nc.scalar.activation(pe[:], p_edge[0:8, :], AF.Exp)
                    # denominator (broadcast over partitions via all-ones lhsT)
                    p_den = psums.tile([128, TT], dt.float32, tag="big")
                    for (c0, cn) in CHUNKS:
                        nc.tensor.matmul(p_den[:, c0:c0 + cn], smalls["allones"][:],
                                         pa[:, c0:c0 + cn], start=True, stop=False)
                        nc.tensor.matmul(p_den[:, c0:c0 + cn], smalls["allones"][0:8],
                                         pe[:, c0:c0 + cn], start=False, stop=True)
                    rec = sbufs.tile([128, TT], dt.bfloat16, tag="rec")
                    with nc.allow_low_precision(reason="softmax denom recip in bf16"):
                        nc.vector.reciprocal(rec[:], p_den[:])
                    # attn value matmuls
                    p_av = psums.tile([128, TT], dt.float32, tag="big")
                    for qt in range(NT):
                        q0 = 128 * qt
                        nc.tensor.matmul(p_av[:, q0:q0 + 128], v_nat[:, qt, 128 * h:128 * h + 128],
                                         pa[:, q0:q0 + 128], start=True, stop=False)
                        nc.tensor.matmul(p_av[:, q0:q0 + 128], v_halo[:, qt, 128 * h:128 * h + 128],
                                         pe[:, q0:q0 + 128], start=False, stop=True)
                    nc.vector.tensor_tensor(attn[:, h], p_av[:], rec[:], ALU.mult)

                # --- Wo projection + residual ---
                rc = acts.tile([128, DT, TT], dt.bfloat16, tag=f"cat{(l + 1) % 2}")
                for o in range(DT):
                    p = psums.tile([128, TT], dt.float32, tag="big")
                    for (c0, cn) in CHUNKS:
                        for d in range(DT):
                            nc.tensor.matmul(p[:, c0:c0 + cn],
                                             w["wo"][:, d, 128 * o:128 * o + 128],
                                             attn[:, d, c0:c0 + cn],
                                             start=(d == 0), stop=False)
                        nc.tensor.matmul(p[:, c0:c0 + cn], smalls["ident"][:],
                                         cat[:, o, c0:c0 + cn], start=False, stop=True)
                    nc.scalar.activation(rc[:, o], p[:], AF.Identity,
                                         bias=bias["bo"][:, o:o + 1], scale=1.0)

                # --- ff_ln -> zf ---
                zf = acts.tile([128, DT, TT], dt.bfloat16, tag="z")
                _ln_normalize(nc, acts, sbufs, psums, smalls, rc, zf, eps_tile)

                # --- FFN ---
                h1 = acts.tile([128, TT], dt.bfloat16, tag="h1")
                p = psums.tile([128, TT], dt.float32, tag="big")
                for (c0, cn) in CHUNKS:
                    for d in range(DT):
                        nc.tensor.matmul(p[:, c0:c0 + cn], w["w1"][:, d, :],
                                         zf[:, d, c0:c0 + cn],
                                         start=(d == 0), stop=(d == DT - 1))
                nc.scalar.activation(h1[:], p[:], AF.Relu, bias=bias["b1"][:], scale=1.0)
                y = acts.tile([128, DT, TT], dt.bfloat16, tag="q")
                for o in range(DT):
                    p = psums.tile([128, TT], dt.float32, tag="big")
                    for (c0, cn) in CHUNKS:
                        nc.tensor.matmul(p[:, c0:c0 + cn], w["w2"][:, 128 * o:128 * o + 128],
                                         h1[:, c0:c0 + cn], start=True, stop=False)
                        nc.tensor.matmul(p[:, c0:c0 + cn], smalls["ident"][:],
                                         rc[:, o, c0:c0 + cn], start=False, stop=True)
                    nc.scalar.activation(y[:, o], p[:], AF.Identity,
                                         bias=bias["b2"][:, o:o + 1], scale=1.0)

                # --- ln_out -> next cat (with affine go/bo2) ---
                cat_next = acts.tile([128, DT, TT], dt.bfloat16, tag=f"cat{(l + 1) % 2}")
                zo = acts.tile([128, DT, TT], dt.bfloat16, tag="z")
                _ln_normalize(nc, acts, sbufs, psums, smalls, y, zo, eps_tile)
                for d in range(DT):
                    nc.vector.tensor_scalar(cat_next[:, d], zo[:, d],
                                            bias["go"][:, d:d + 1], bias["bo2"][:, d:d + 1],
                                            ALU.mult, ALU.add)
                cat = cat_next

            # --- mean-pool utterance tokens (pos 1..4 of each 5-block) ---
            out_sb = sbufs.tile([128, DT], dt.float32, tag="outsb")
            for d in range(DT):
                view = cat[:, d, :].rearrange("p (s j) -> p s j", j=5)[:, :, 1:5]
                nc.vector.tensor_reduce(out_sb[:, d:d + 1], view,
                                        axis=mybir.AxisListType.XY, op=ALU.add)
            nc.vector.tensor_scalar_mul(out_sb[:], out_sb[:], 1.0 / U)
            nc.sync.dma_start(out_dram, out_sb[:])
    nc.compile()
    return nc


def _build():
    nc = bacc.Bacc("TRN2", target_bir_lowering=False, debug=False, num_devices=NCORES)
    return _trace(nc)


def kernel(**inputs):
    global _COMPILED
    ins = {k: np.asarray(v) for k, v in inputs.items()}
    shared = _host_prep(ins)
    idx = _tok_index()
    x = ins["x"].astype(np.float32)          # [B, T, D]
    xp = x[:, idx, :]                        # [B, TT, D]
    xT = np.ascontiguousarray(xp.transpose(0, 2, 1)).astype(bf16)  # [B, D, TT]
    if _COMPILED is None:
        _COMPILED = _build()
    nc = _COMPILED
    in_maps = []
    for b in range(NCORES):
        m = dict(shared)
        m["xT"] = xT[b]
        in_maps.append(m)
    res = bass_utils.run_bass_kernel_spmd(nc, in_maps, core_ids=list(range(NCORES)))
    outs = []
    for b in range(NCORES):
        o = res.results[b]["out"]            # [128, DT]
        outs.append(o.T.reshape(D))          # d = dtile*128 + p
    return np.stack(outs).astype(np.float32)



# revision 3
# speedup vs baseline: 1.8344x; 1.8344x over previous
"""Trainium2 Bass kernel for nn_BasicNet4 (Emformer encoder, sparse attention).

Strategy (v2):
  - Data-parallel over batch B=8 across 8 NeuronCores (weights replicated).
  - Tokens reordered host-side into segment-interleaved order:
    seg i -> [rc_i, u_{4i}, u_{4i+1}, u_{4i+2}, u_{4i+3}]  (5 tokens x 256 segs = 1280)
    so attention is block-diagonal with 5x5 blocks.
  - Attention computed in 125-token query windows (25 whole segments) with a
    128-token key window starting at the same offset: every window has an
    IDENTICAL block-diagonal mask, no edge/halo handling at all.
  - Activations transposed in SBUF: [d on partitions (4 tiles of 128), tokens
    on free].  LN stats via ones-matmul partition reductions (broadcast form).
  - "s-stream" reparameterization: when ln_out gain/bias are scalar (they are
    for this model: gamma=1, beta=0), the residual stream is kept LN-normalized
    so ln_in of layers 1..3 is a no-op (folded into the QKV weights), and the
    ln_out affine is folded into Wo/W2/bo/b2 of the next layer + a final
    host-side affine.
  - V bias folded into the Wo bias (attention rows sum to 1), so the V
    projection has no bias pass.
  - Residual adds + biases fused into single DVE scalar_tensor_tensor ops
    (no identity-matmul residuals).
  - Softmax reciprocal via reciprocal_approx_fast (5x faster than the DVE
    reciprocal that dominated the old critical path).
  - bf16 matmul operands / residual stream, fp32 PSUM accumulation.
"""

import sys

sys.path.insert(0, "/opt/trn_rl_repo")

import numpy as np
import ml_dtypes

import concourse.bass as bass
import concourse.mybir as mybir
import concourse.tile as tile
from concourse import bass_utils, bacc

bf16 = ml_dtypes.bfloat16
dt = mybir.dt
AF = mybir.ActivationFunctionType
ALU = mybir.AluOpType

# Model config (hardcoded from the problem spec)
D, H, FFN, L = 512, 4, 128, 4
SEG, RC = 4, 1
B, T = 8, 1025
U = T - RC            # 1024
NSEG = U // SEG       # 256
TT = NSEG * (SEG + RC)  # 1280 interleaved tokens
DT = D // 128         # 4 d tiles
DH = D // H           # 128 (= one partition tile per head)
NCORES = 8
CHUNKS = [(0, 512), (512, 512), (1024, 256)]  # free-dim chunks <= 512 (PSUM bank)

WQ = 125              # query-window stride (25 whole segments)
KW = 128              # key-window width
NW = -(-TT // WQ)     # 11 windows (last one is 30 tokens)
RANK = 1 + (KW // 5)  # 26: mask factorization rank (1 bias row + 25 segs)
# window groups of <=4 windows -> one 512-col PSUM tile each
GROUPS = [list(range(4 * g, min(4 * g + 4, NW))) for g in range(-(-NW // 4))]
PADW = 128 * NW       # padded col space (128 cols per window)

CBF = np.float32(bf16(np.float32(1e9)))  # mask constant, exact in bf16

_COMPILED = None
_FAST = None


def _tok_index():
    # interleaved token t -> original frame index in x[:, :T]
    t = np.arange(TT)
    seg = t // 5
    pos = t % 5
    off = np.array([4, 0, 1, 2, 3])[pos]
    return 4 * seg + off  # in [0, 1024]


def _win_geom(w):
    q0 = WQ * w
    qn = min(KW, TT - q0)   # query stream width (masked beyond 125)
    kn = min(KW, TT - q0)   # key window width
    return q0, qn, kn


def _mask_consts():
    """lmask [RANK,128] (lhsT), rmask [RANK,128*NW] (rhs):
    sum_r lmask[r,m]*rmask[r, 128w+j] = -C + C*[m//5 == j//5] for real in-window
    query cols j<125 (and j within bounds), -C for pad/overhang cols."""
    lm = np.zeros((RANK, KW), np.float32)
    lm[0, :] = 1.0
    segk = np.arange(KW) // 5          # 0..25 (seg 25 has no indicator row)
    for i in range(25):
        lm[1 + i, :] = (segk == i)
    rm = np.zeros((RANK, 128 * NW), np.float32)
    for w in range(NW):
        q0, qn, _ = _win_geom(w)
        nreal = min(WQ, TT - q0)       # real query cols in this window
        col = 128 * w
        rm[0, col:col + 128] = -CBF
        for j in range(nreal):
            rm[1 + (j // 5), col + j] = CBF
    return lm.astype(bf16), rm.astype(bf16)


def _fast_ok(ins):
    """Fast path: ln_out gain/bias scalar (and gain>0) for layers 0..L-2."""
    f32 = np.float32
    for l in range(L - 1):
        g = f32(ins["ln_out_g"][l])
        b = f32(ins["ln_out_b"][l])
        if not (np.all(g == g[0]) and g[0] > 0 and np.all(b == b[0])):
            return False
    return True


def _host_prep(ins, fast):
    """Fold LN affines/scales into weights, transpose, cast. Shared input map."""
    f32 = np.float32
    m = {}
    scale = np.float32(DH) ** -0.5
    for l in range(L):
        g_i, b_i = f32(ins["ln_in_g"][l]), f32(ins["ln_in_b"][l])
        g_f, b_f = f32(ins["ff_ln_g"][l]), f32(ins["ff_ln_b"][l])
        Wq = f32(ins["Wq"][l]);  bq = f32(ins["bq"][l])
        Wk = f32(ins["Wkv"][l][:D]);  bk = f32(ins["bkv"][l][:D])
        Wv = f32(ins["Wkv"][l][D:]);  bv = f32(ins["bkv"][l][D:])
        Wo = f32(ins["Wo"][l]);  bo = f32(ins["bo"][l])
        W1 = f32(ins["W1"][l]);  b1 = f32(ins["b1"][l])
        W2 = f32(ins["W2"][l]);  b2 = f32(ins["b2"][l])
        gp = f32(1.0)
        if fast and l > 0:
            gp = f32(ins["ln_out_g"][l - 1][0])   # scalar, >0 (checked)
        Wq_ = scale * (Wq * g_i[None, :]); bq_ = scale * (bq + Wq @ b_i)
        Wk_ = Wk * g_i[None, :];           bk_ = bk + Wk @ b_i
        Wv_ = Wv * g_i[None, :];           bv_ = bv + Wv @ b_i
        Wo_ = Wo / gp;                     bo_ = (bo + Wo @ bv_) / gp
        W1_ = W1 * g_f[None, :];           b1_ = b1 + W1 @ b_f
        W2_ = W2 / gp;                     b2_ = b2 / gp
        m[f"wq{l}"] = Wq_.T.copy().astype(bf16)   # [din, dout]
        m[f"wk{l}"] = Wk_.T.copy().astype(bf16)
        m[f"wv{l}"] = Wv_.T.copy().astype(bf16)
        m[f"wo{l}"] = Wo_.T.copy().astype(bf16)
        m[f"w1{l}"] = W1_.T.copy().astype(bf16)   # [512, 128]
        m[f"w2{l}"] = W2_.T.copy().astype(bf16)   # [128, 512]
        m[f"bq{l}"] = bq_.reshape(DT, 128).T.copy()       # [128, DT] f32
        m[f"bk{l}"] = bk_.reshape(DT, 128).T.copy()
        m[f"bo{l}"] = bo_.reshape(DT, 128).T.copy()
        m[f"b1{l}"] = b1_.reshape(1, FFN).T.copy()        # [128, 1]
        m[f"b2{l}"] = b2_.reshape(DT, 128).T.copy()
        if not fast:
            m[f"go{l}"] = f32(ins["ln_out_g"][l]).reshape(DT, 128).T.copy()
            m[f"bo2{l}"] = f32(ins["ln_out_b"][l]).reshape(DT, 128).T.copy()
    lm, rm = _mask_consts()
    m["lmask"] = lm
    m["rmask"] = rm
    m["ones_c"] = np.full((128, 128), 1.0 / D, bf16)  # stats lhsT (bcast mean)
    m["allones"] = np.ones((128, 128), bf16)          # softmax denominator lhsT
    return m


def _dram_inputs(nc, fast):
    a = {}
    def inp(name, shape, dtype):
        a[name] = nc.dram_tensor(name, list(shape), dtype, kind="ExternalInput").ap()
    inp("xT", (D, TT), dt.bfloat16)
    for l in range(L):
        inp(f"wq{l}", (D, D), dt.bfloat16); inp(f"wk{l}", (D, D), dt.bfloat16)
        inp(f"wv{l}", (D, D), dt.bfloat16); inp(f"wo{l}", (D, D), dt.bfloat16)
        inp(f"w1{l}", (D, FFN), dt.bfloat16); inp(f"w2{l}", (FFN, D), dt.bfloat16)
        inp(f"bq{l}", (128, DT), dt.float32); inp(f"bk{l}", (128, DT), dt.float32)
        inp(f"bo{l}", (128, DT), dt.float32)
        inp(f"b1{l}", (128, 1), dt.float32); inp(f"b2{l}", (128, DT), dt.float32)
        if not fast:
            inp(f"go{l}", (128, DT), dt.float32)
            inp(f"bo2{l}", (128, DT), dt.float32)
    inp("lmask", (RANK, KW), dt.bfloat16)
    inp("rmask", (RANK, 128 * NW), dt.bfloat16)
    inp("ones_c", (128, 128), dt.bfloat16)
    inp("allones", (128, 128), dt.bfloat16)
    out = nc.dram_tensor("out", [128, DT], dt.float32, kind="ExternalOutput").ap()
    return a, out


def _trace(nc, fast):
    a, out_dram = _dram_inputs(nc, fast)
    with tile.TileContext(nc) as tc:
        import contextlib
        ctx = contextlib.ExitStack()
        with ctx:
            consts = ctx.enter_context(tc.tile_pool(name="consts", bufs=1))
            wpool = ctx.enter_context(tc.tile_pool(name="w", bufs=2))
            acts = ctx.enter_context(tc.tile_pool(name="acts", bufs=1))
            small = ctx.enter_context(tc.tile_pool(name="small", bufs=2))
            psum = ctx.enter_context(tc.tile_pool(name="psum", bufs=1, space="PSUM"))

            # ---- constants ----
            smalls = {}
            for name, shape, dd in [
                ("lmask", [RANK, KW], dt.bfloat16),
                ("rmask", [RANK, 128 * NW], dt.bfloat16),
                ("ones_c", [128, 128], dt.bfloat16),
                ("allones", [128, 128], dt.bfloat16),
            ]:
                t = consts.tile(shape, dd, tag=name)
                nc.sync.dma_start(t[:], a[name])
                smalls[name] = t
            eps_tile = consts.tile([128, 1], dt.float32)
            nc.vector.memset(eps_tile[:], 1e-5)
            ones_c, allones = smalls["ones_c"], smalls["allones"]
            lmask, rmask = smalls["lmask"], smalls["rmask"]

            def ln(src, dst, apply_A):
                """dst = src - mean  (and * rstd when apply_A).  Returns the
                per-token rstd tile A [128, TT] f32 (broadcast rows)."""
                sq = acts.tile([128, DT, TT], dt.bfloat16, tag="sq")
                for d in range(DT):
                    nc.gpsimd.tensor_tensor(sq[:, d], src[:, d], src[:, d], ALU.mult)
                A = acts.tile([128, TT], dt.float32, tag="A", bufs=3)
                for (c0, cn) in CHUNKS:
                    pmu = psum.tile([128, 512], dt.float32, tag="pp", bufs=4)
                    pe2 = psum.tile([128, 512], dt.float32, tag="pp", bufs=4)
                    for d in range(DT):
                        nc.tensor.matmul(pmu[:, :cn], ones_c[:], src[:, d, c0:c0 + cn],
                                         start=(d == 0), stop=(d == DT - 1))
                    for d in range(DT):
                        nc.tensor.matmul(pe2[:, :cn], ones_c[:], sq[:, d, c0:c0 + cn],
                                         start=(d == 0), stop=(d == DT - 1))
                    sqmu = small.tile([128, 512], dt.float32, tag="sqmu")
                    nc.scalar.activation(sqmu[:, :cn], pmu[:, :cn], AF.Square)
                    var = small.tile([128, 512], dt.float32, tag="var")
                    nc.vector.tensor_tensor(var[:, :cn], pe2[:, :cn], sqmu[:, :cn],
                                            ALU.subtract)
                    sd = small.tile([128, 512], dt.float32, tag="sd")
                    nc.scalar.activation(sd[:, :cn], var[:, :cn], AF.Sqrt,
                                         bias=eps_tile[:], scale=1.0)
                    nc.vector.reciprocal_approx_fast(A[:, c0:c0 + cn], sd[:, :cn])
                    for d in range(DT):
                        if apply_A:
                            t2 = small.tile([128, 512], dt.bfloat16, tag="t2")
                            nc.vector.scalar_tensor_tensor(
                                t2[:, :cn], pmu[:, :cn], -1.0, src[:, d, c0:c0 + cn],
                                ALU.mult, ALU.add)
                            nc.vector.tensor_tensor(dst[:, d, c0:c0 + cn], t2[:, :cn],
                                                    A[:, c0:c0 + cn], ALU.mult)
                        else:
                            nc.vector.scalar_tensor_tensor(
                                dst[:, d, c0:c0 + cn], pmu[:, :cn], -1.0,
                                src[:, d, c0:c0 + cn], ALU.mult, ALU.add)
                return A

            # ---- initial residual: raw x (interleaved, transposed) ----
            res = acts.tile([128, DT, TT], dt.bfloat16, tag="res", bufs=2)
            nc.sync.dma_start(res[:], a["xT"].rearrange("(dtile p) t -> p dtile t", p=128))

            for l in range(L):
                # ---- layer weights ----
                w = {}
                for nm, shape in [("wq", [128, DT, D]), ("wk", [128, DT, D]),
                                  ("wv", [128, DT, D]), ("wo", [128, DT, D]),
                                  ("w1", [128, DT, FFN])]:
                    t = wpool.tile(shape, dt.bfloat16, tag=nm)
                    nc.sync.dma_start(t[:], a[f"{nm}{l}"].rearrange(
                        "(dtile p) o -> p dtile o", p=128))
                    w[nm] = t
                w["w2"] = wpool.tile([128, D], dt.bfloat16, tag="w2", name="w2")
                nc.sync.dma_start(w["w2"][:], a[f"w2{l}"])
                bias = {}
                bnames = ["bq", "bk", "bo", "b1", "b2"] + ([] if fast else ["go", "bo2"])
                for nm in bnames:
                    t = wpool.tile([128, DT] if nm != "b1" else [128, 1],
                                   dt.float32, tag=nm)
                    nc.sync.dma_start(t[:], a[f"{nm}{l}"])
                    bias[nm] = t

                # ---- ln_in (explicit for layer 0 / general path) ----
                if l == 0 or not fast:
                    zq = acts.tile([128, DT, TT], dt.bfloat16, tag="zc", bufs=2)
                    ln(res, zq, apply_A=True)
                else:
                    zq = res

                # ---- Q, K projections (weights stationary, transposed out) ----
                qk = {}
                for nm, bnm, tg in [("wq", "bq", "qt"), ("wk", "bk", "kt")]:
                    dst = acts.tile([128, DT, TT], dt.bfloat16, tag=tg)
                    for o in range(DT):
                        for (c0, cn) in CHUNKS:
                            p = psum.tile([128, 512], dt.float32, tag="pp", bufs=4)
                            for d in range(DT):
                                nc.tensor.matmul(
                                    p[:, :cn],
                                    w[nm][:, d, 128 * o:128 * o + 128],
                                    zq[:, d, c0:c0 + cn],
                                    start=(d == 0), stop=(d == DT - 1))
                            nc.scalar.activation(dst[:, o, c0:c0 + cn], p[:, :cn],
                                                 AF.Identity,
                                                 bias=bias[bnm][:, o:o + 1], scale=1.0)
                    qk[nm] = dst
                q_t, k_t = qk["wq"], qk["wk"]

                # ---- V in overlapping 128-token key windows (no bias: folded) ----
                v_win = acts.tile([128, NW, D], dt.bfloat16, tag="vw")
                for wi in range(NW):
                    kw0, _, kn = _win_geom(wi)
                    p = psum.tile([128, 512], dt.float32, tag="pp", bufs=4)
                    for d in range(DT):
                        nc.tensor.matmul(p[0:kn, :], zq[:, d, kw0:kw0 + kn],
                                         w["wv"][:, d, :],
                                         start=(d == 0), stop=(d == DT - 1))
                    nc.scalar.activation(v_win[0:kn, wi, :], p[0:kn, :], AF.Copy)

                # ---- attention: per head, per 4-window group ----
                attn = acts.tile([128, DT, TT], dt.bfloat16, tag="at")
                for h in range(H):
                    for g, wlist in enumerate(GROUPS):
                        ng = 128 * len(wlist)
                        ps = psum.tile([128, 512], dt.float32, tag="ps", bufs=2)
                        nc.tensor.matmul(ps[:, :ng], lmask[:],
                                         rmask[:, 512 * g:512 * g + ng],
                                         start=True, stop=False)
                        for wi in wlist:
                            q0, qn, kn = _win_geom(wi)
                            ow = 128 * (wi - wlist[0])
                            nc.tensor.matmul(ps[0:kn, ow:ow + qn],
                                             k_t[:, h, q0:q0 + kn],
                                             q_t[:, h, q0:q0 + qn],
                                             start=False, stop=True)
                        pa = small.tile([128, 512], dt.bfloat16, tag="pa", bufs=3)
                        nc.scalar.activation(pa[:, :ng], ps[:, :ng], AF.Exp)
                        pd = psum.tile([128, 512], dt.float32, tag="pd", bufs=1)
                        nc.tensor.matmul(pd[:, :ng], allones[:], pa[:, :ng],
                                         start=True, stop=True)
                        rec = small.tile([128, 512], dt.float32, tag="rec")
                        nc.vector.reciprocal_approx_fast(rec[:, :ng], pd[:, :ng])
                        pav = psum.tile([128, 512], dt.float32, tag="pav", bufs=1)
                        for wi in wlist:
                            q0, qn, kn = _win_geom(wi)
                            ow = 128 * (wi - wlist[0])
                            nc.tensor.matmul(pav[:, ow:ow + qn],
                                             v_win[0:kn, wi, 128 * h:128 * h + 128],
                                             pa[0:kn, ow:ow + qn],
                                             start=True, stop=True)
                        # normalize + compact (drop per-window pad/overhang cols)
                        full = [wi for wi in wlist if WQ * wi + WQ <= TT]
                        nf = len(full)
                        qg0 = WQ * wlist[0]
                        if nf:
                            pav_v = pav[:, :].rearrange("p (w j) -> p w j", j=128)
                            rec_v = rec[:, :].rearrange("p (w j) -> p w j", j=128)
                            nc.vector.tensor_tensor(
                                attn[:, h, qg0:qg0 + WQ * nf],
                                pav_v[:, 0:nf, 0:WQ], rec_v[:, 0:nf, 0:WQ], ALU.mult)
                        for wi in wlist[nf:]:          # partial tail window
                            q0, qn, _ = _win_geom(wi)
                            ow = 128 * (wi - wlist[0])
                            nc.vector.tensor_tensor(
                                attn[:, h, q0:TT],
                                pav[:, ow:ow + (TT - q0)],
                                rec[:, ow:ow + (TT - q0)], ALU.mult)

                # ---- Wo projection + bias + residual (fused drain) ----
                rc = acts.tile([128, DT, TT], dt.bfloat16, tag="rc")
                for o in range(DT):
                    for (c0, cn) in CHUNKS:
                        p = psum.tile([128, 512], dt.float32, tag="pp", bufs=4)
                        for d in range(DT):
                            nc.tensor.matmul(p[:, :cn],
                                             w["wo"][:, d, 128 * o:128 * o + 128],
                                             attn[:, d, c0:c0 + cn],
                                             start=(d == 0), stop=(d == DT - 1))
                        nc.vector.scalar_tensor_tensor(
                            rc[:, o, c0:c0 + cn], p[:, :cn], bias["bo"][:, o:o + 1],
                            res[:, o, c0:c0 + cn], ALU.add, ALU.add)

                # ---- ff_ln: center only; rstd folded into post-W1 scale ----
                zc = acts.tile([128, DT, TT], dt.bfloat16, tag="zc", bufs=2)
                A1 = ln(rc, zc, apply_A=False)

                # ---- FFN ----
                h1 = acts.tile([128, TT], dt.bfloat16, tag="h1")
                for (c0, cn) in CHUNKS:
                    p = psum.tile([128, 512], dt.float32, tag="pp", bufs=4)
                    for d in range(DT):
                        nc.tensor.matmul(p[:, :cn], w["w1"][:, d, :],
                                         zc[:, d, c0:c0 + cn],
                                         start=(d == 0), stop=(d == DT - 1))
                    t1 = small.tile([128, 512], dt.bfloat16, tag="t1")
                    nc.vector.tensor_tensor(t1[:, :cn], p[:, :cn], A1[:, c0:c0 + cn],
                                            ALU.mult)
                    nc.scalar.activation(h1[:, c0:c0 + cn], t1[:, :cn], AF.Relu,
                                         bias=bias["b1"][:], scale=1.0)
                y = acts.tile([128, DT, TT], dt.bfloat16, tag="y")
                for o in range(DT):
                    for (c0, cn) in CHUNKS:
                        p = psum.tile([128, 512], dt.float32, tag="pp", bufs=4)
                        nc.tensor.matmul(p[:, :cn], w["w2"][:, 128 * o:128 * o + 128],
                                         h1[:, c0:c0 + cn], start=True, stop=True)
                        nc.vector.scalar_tensor_tensor(
                            y[:, o, c0:c0 + cn], p[:, :cn], bias["b2"][:, o:o + 1],
                            rc[:, o, c0:c0 + cn], ALU.add, ALU.add)

                # ---- ln_out -> next residual (normalized s-stream) ----
                s_next = acts.tile([128, DT, TT], dt.bfloat16, tag="res", bufs=2)
                ln(y, s_next, apply_A=True)
                if not fast:
                    for d in range(DT):
                        nc.vector.tensor_scalar(s_next[:, d], s_next[:, d],
                                                bias["go"][:, d:d + 1],
                                                bias["bo2"][:, d:d + 1],
                                                ALU.mult, ALU.add)
                res = s_next

            # ---- mean-pool utterance tokens (pos 1..4 of each 5-block) ----
            out_sb = small.tile([128, DT], dt.float32, tag="outsb")
            for d in range(DT):
                view = res[:, d, :].rearrange("p (s j) -> p s j", j=5)[:, :, 1:5]
                nc.vector.tensor_reduce(out_sb[:, d:d + 1], view,
                                        axis=mybir.AxisListType.XY, op=ALU.add)
            nc.vector.tensor_scalar_mul(out_sb[:], out_sb[:], 1.0 / U)
            nc.sync.dma_start(out_dram, out_sb[:])
    nc.compile()
    return nc


def _build(fast):
    nc = bacc.Bacc("TRN2", target_bir_lowering=False, debug=False, num_devices=NCORES)
    return _trace(nc, fast)


def kernel(**inputs):
    global _COMPILED, _FAST
    ins = {k: np.asarray(v) for k, v in inputs.items()}
    fast = _fast_ok(ins)
    shared = _host_prep(ins, fast)
    idx = _tok_index()
    x = ins["x"].astype(np.float32)          # [B, T, D]
    xp = x[:, idx, :]                        # [B, TT, D]
    xT = np.ascontiguousarray(xp.transpose(0, 2, 1)).astype(bf16)  # [B, D, TT]
    if _COMPILED is None or _FAST != fast:
        _COMPILED = _build(fast)
        _FAST = fast
    nc = _COMPILED
    in_maps = []
    for b in range(NCORES):
        m = dict(shared)
        m["xT"] = xT[b]
        in_maps.append(m)
    res = bass_utils.run_bass_kernel_spmd(nc, in_maps, core_ids=list(range(NCORES)))
    outs = []
    for b in range(NCORES):
        o = res.results[b]["out"]            # [128, DT]
        outs.append(o.T.reshape(D))          # d = dtile*128 + p
    out = np.stack(outs).astype(np.float32)
    if fast:
        g3 = np.float32(ins["ln_out_g"][L - 1])
        b3 = np.float32(ins["ln_out_b"][L - 1])
        out = out * g3[None, :] + b3[None, :]
    return out
